# revision 7
# baseline (speedup 1.0000x reference)
"""BEVPool (segment-sum) Trainium2 kernel.

Sharding: Nprime points split contiguously across 8 NeuronCores.
Per core: compute voxel cell per point (reciprocal-multiply floor, bit-exact
vs the jax-on-neuron reference), group cells into "quad rows" (4 cells = one
1KB row so the whole 360x360 grid fits int16 row indexing), deduplicate
quad rows within each 128-token tile via a PE equality-matrix matmul (the
matmul also performs the in-tile aggregation), then dma_scatter_add each
tile's unique rows into round-robin DRAM grids (cross-call ordering is
serialized per grid, so accumulation is exact f32). Host sums partial grids.
"""

import numpy as np

import concourse.bacc as bacc
import concourse.bass as bass
import concourse.mybir as mybir
from concourse import tile
from concourse.bass_utils import run_bass_kernel_spmd

f32 = mybir.dt.float32
i16 = mybir.dt.int16
i32 = mybir.dt.int32
Op = mybir.AluOpType
AX = mybir.AxisListType

NP_TOTAL = 1 * 6 * 118 * 32 * 88          # 1993728 points
NCORES = 8
NP_CORE = NP_TOTAL // NCORES              # 249216 = 128 * 1947
C = 64
H = W = 360
NCELL = H * W                             # 129600
NQUAD = NCELL // 4                        # 32400 quad rows (4 cells each)
GARB = NQUAD                              # garbage quad row
NGRID = 3                                 # round-robin output grids
CHUNK_TILES = 64                          # tiles per chunk (8192 tokens)

RECIP = float(np.float32(np.float32(1.0) / np.float32(0.3)))

_cache = {}


def build_program(np_core=NP_CORE, ncores=NCORES):
    ntiles = np_core // 128
    nc = bacc.Bacc("TRN2", target_bir_lowering=False, debug=False,
                   num_devices=ncores)
    geom_d = nc.dram_tensor("geom", [np_core, 3], f32, kind="ExternalInput")
    x_d = nc.dram_tensor("x", [np_core, C], f32, kind="ExternalInput")
    grids = [
        nc.dram_tensor(f"grid{g}", [NQUAD + 1, 4 * C], f32,
                       kind="ExternalOutput")
        for g in range(NGRID)
    ]

    geom_ap = geom_d.ap()
    x_ap = x_d.ap()

    with tile.TileContext(nc) as tc:
        with (
            tc.tile_pool(name="const", bufs=1) as cpool,
            tc.tile_pool(name="work", bufs=2) as pool,
            tc.tile_pool(name="tiny", bufs=4) as tpool,
            tc.tile_pool(name="psd", bufs=2, space="PSUM") as ppoolD,
            tc.tile_pool(name="psa", bufs=2, space="PSUM") as ppoolA,
        ):
            iota_i = cpool.tile([128, 128], i32, tag="iota_i")
            nc.gpsimd.iota(iota_i[:], [[1, 128]], channel_multiplier=0)
            iota_f = cpool.tile([128, 128], f32, tag="iota_f")
            nc.vector.tensor_copy(iota_f[:], iota_i[:])
            pidx_i = cpool.tile([128, 1], i32, tag="pidx_i")
            nc.gpsimd.iota(pidx_i[:], [[0, 1]], channel_multiplier=1)
            pidx = cpool.tile([128, 1], f32, tag="pidx")
            nc.vector.tensor_copy(pidx[:], pidx_i[:])
            ident = cpool.tile([128, 128], f32, tag="ident")
            nc.vector.tensor_scalar(ident[:], iota_f[:], pidx[:], None,
                                    Op.is_equal)
            ltri = cpool.tile([128, 128], f32, tag="ltri")
            nc.vector.tensor_scalar(ltri[:], iota_f[:], pidx[:], None,
                                    Op.is_lt)
            onesrow = cpool.tile([1, 128], f32, tag="onesrow")
            nc.vector.memset(onesrow[:], 1.0)

            tile_no = 0
            done = 0
            while done < ntiles:
                nt = min(CHUNK_TILES, ntiles - done)
                tok0 = done * 128
                ntok = nt * 128
                # ---- load chunk (token i -> partition i%? : contiguous:
                # partition p holds tokens [p*nt, (p+1)*nt) of the chunk) ----
                xt = pool.tile([128, CHUNK_TILES * C], f32, tag="xt")
                nc.sync.dma_start(
                    xt[:, :nt * C],
                    x_ap[tok0:tok0 + ntok, :].rearrange(
                        "(p t) c -> p (t c)", p=128),
                )
                gt = pool.tile([128, CHUNK_TILES * 3], f32, tag="gt")
                nc.sync.dma_start(
                    gt[:, :nt * 3],
                    geom_ap[tok0:tok0 + ntok, :].rearrange(
                        "(p t) c -> p (t c)", p=128),
                )

                # ---- cell math ----
                def floordiv(coord_ap, tag):
                    w = pool.tile([128, CHUNK_TILES], f32, tag=tag + "w")
                    nc.vector.tensor_scalar(w[:, :nt], coord_ap, 54.0, RECIP,
                                            Op.add, Op.mult)
                    giq = pool.tile([128, CHUNK_TILES], i32, tag=tag + "i")
                    nc.vector.tensor_copy(giq[:, :nt], w[:, :nt])
                    gf = pool.tile([128, CHUNK_TILES], f32, tag=tag + "f")
                    nc.vector.tensor_copy(gf[:, :nt], giq[:, :nt])
                    d = pool.tile([128, CHUNK_TILES], f32, tag=tag + "d")
                    nc.vector.tensor_tensor(d[:, :nt], gf[:, :nt], w[:, :nt],
                                            Op.is_gt)
                    g = pool.tile([128, CHUNK_TILES], f32, tag=tag + "g")
                    nc.vector.tensor_tensor(g[:, :nt], gf[:, :nt], d[:, :nt],
                                            Op.subtract)
                    return g

                gx = floordiv(gt[:, 0:nt * 3:3], "gx")
                gy = floordiv(gt[:, 1:nt * 3:3], "gy")
                cell = pool.tile([128, CHUNK_TILES], f32, tag="cell")
                nc.vector.tensor_scalar(cell[:, :nt], gx[:, :nt], 360.0, None,
                                        Op.mult)
                nc.vector.tensor_tensor(cell[:, :nt], cell[:, :nt],
                                        gy[:, :nt], Op.add)
                nc.vector.tensor_scalar(cell[:, :nt], cell[:, :nt], 0.0,
                                        float(NCELL - 1), Op.max, Op.min)
                quad = pool.tile([128, CHUNK_TILES], f32, tag="quad")
                qi = pool.tile([128, CHUNK_TILES], i32, tag="qi")
                qtrue = pool.tile([128, CHUNK_TILES], f32, tag="qtrue")
                nc.vector.tensor_scalar(qtrue[:, :nt], cell[:, :nt], 0.25,
                                        None, Op.mult)
                nc.vector.tensor_copy(qi[:, :nt], qtrue[:, :nt])
                nc.vector.tensor_copy(quad[:, :nt], qi[:, :nt])
                qd = pool.tile([128, CHUNK_TILES], f32, tag="qd")
                nc.vector.tensor_tensor(qd[:, :nt], quad[:, :nt],
                                        qtrue[:, :nt], Op.is_gt)
                nc.vector.tensor_tensor(quad[:, :nt], quad[:, :nt],
                                        qd[:, :nt], Op.subtract)
                r4 = pool.tile([128, CHUNK_TILES], f32, tag="r4")
                nc.vector.tensor_scalar(r4[:, :nt], quad[:, :nt], -4.0, None,
                                        Op.mult)
                nc.vector.tensor_tensor(r4[:, :nt], r4[:, :nt], cell[:, :nt],
                                        Op.add)
                masks = []
                for s in range(4):
                    m = pool.tile([128, CHUNK_TILES], f32, tag=f"m{s}")
                    nc.vector.tensor_scalar(m[:, :nt], r4[:, :nt], float(s),
                                            None, Op.is_equal)
                    masks.append(m)

                rankarr = pool.tile([128, CHUNK_TILES], f32, tag="rankarr")

                def emat(T, tag):
                    """Equality matrix E[i,j] = (quad_i == quad_j), SBUF."""
                    qcol = quad[:, T:T + 1]
                    psTt = ppoolD.tile([128, 128], f32, tag="psT")
                    psT = psTt[0:1, :]
                    nc.tensor.matmul(psT, qcol, ident[:])
                    qrow = tpool.tile([1, 128], f32, tag="qrow")
                    nc.vector.tensor_copy(qrow[:], psT)
                    nqrow = tpool.tile([1, 128], f32, tag="nqrow")
                    nc.vector.tensor_scalar(nqrow[:], psT, -1.0, None,
                                            Op.mult)
                    psD = ppoolD.tile([128, 128], f32, tag="psD")
                    nc.tensor.matmul(psD[:], qrow[:], onesrow[:],
                                     start=True, stop=False)
                    nc.tensor.matmul(psD[:], onesrow[:], nqrow[:],
                                     start=False, stop=True)
                    E = tpool.tile([128, 128], f32, tag="E")
                    nc.vector.tensor_scalar(E[:], psD[:], 0.0, None,
                                            Op.is_equal)
                    return E

                # ---- phase A: ranks ----
                for T in range(nt):
                    E = emat(T, "a")
                    Elt = tpool.tile([128, 128], f32, tag="Elt")
                    nc.vector.tensor_tensor(Elt[:], E[:], ltri[:], Op.mult)
                    nc.vector.tensor_reduce(rankarr[:, T:T + 1], Elt[:],
                                            AX.X, Op.add)

                # ---- idx select + fold to 16-wrap int16 ----
                idxf = pool.tile([128, CHUNK_TILES], f32, tag="idxf")
                isz = pool.tile([128, CHUNK_TILES], f32, tag="isz")
                nc.vector.tensor_scalar(isz[:, :nt], rankarr[:, :nt], 0.0,
                                        None, Op.is_equal)
                nc.vector.tensor_scalar(idxf[:, :nt], quad[:, :nt],
                                        float(GARB), None, Op.subtract)
                nc.vector.tensor_tensor(idxf[:, :nt], idxf[:, :nt],
                                        isz[:, :nt], Op.mult)
                nc.vector.tensor_scalar(idxf[:, :nt], idxf[:, :nt],
                                        float(GARB), None, Op.add)
                idxt = pool.tile([128, CHUNK_TILES * 8], i16, tag="idxt")
                for r in range(8):
                    nc.gpsimd.dma_start(
                        idxt[0:16, r:8 * nt:8],
                        idxf[16 * r:16 * r + 16, :nt])
                for g8 in range(1, 8):
                    nc.gpsimd.dma_start(
                        idxt[16 * g8:16 * g8 + 16, :8 * nt],
                        idxt[0:16, :8 * nt])

                # ---- phase C: payload, aggregate, scatter ----
                for T in range(nt):
                    E = emat(T, "c")
                    pay = tpool.tile([128, 4 * C], f32, tag="pay")
                    for s in range(4):
                        nc.vector.tensor_scalar(
                            pay[:, s * C:(s + 1) * C],
                            xt[:, T * C:(T + 1) * C],
                            masks[s][:, T:T + 1], None, Op.mult)
                    psA = ppoolA.tile([128, 4 * C], f32, tag="psA")
                    nc.tensor.matmul(psA[:], E[:], pay[:])
                    agg = tpool.tile([128, 4 * C], f32, tag="agg")
                    nc.vector.tensor_copy(agg[:], psA[:])
                    nc.gpsimd.dma_scatter_add(
                        grids[tile_no % NGRID].ap(),
                        agg[:].rearrange("p (b e) -> p b e", b=1),
                        idxt[:, 8 * T:8 * T + 8],
                        128, 128, 4 * C,
                    )
                    tile_no += 1
                done += nt

    nc.compile()
    return nc


def kernel(geom_feats: np.ndarray, x: np.ndarray) -> np.ndarray:
    geom_feats = np.ascontiguousarray(geom_feats, dtype=np.float32)
    x = np.ascontiguousarray(x, dtype=np.float32)
    g2 = geom_feats.reshape(NP_TOTAL, 3)
    x2 = x.reshape(NP_TOTAL, C)

    if "nc" not in _cache:
        _cache["nc"] = build_program()
    nc = _cache["nc"]

    in_maps = []
    for c in range(NCORES):
        sl = slice(c * NP_CORE, (c + 1) * NP_CORE)
        in_maps.append({"geom": g2[sl], "x": x2[sl]})

    res = run_bass_kernel_spmd(nc, in_maps, core_ids=list(range(NCORES)))

    total = np.zeros((NQUAD, 4 * C), np.float64)
    for c in range(NCORES):
        for g in range(NGRID):
            total += res.results[c][f"grid{g}"][:NQUAD].astype(np.float64)
    cells = total.reshape(NQUAD * 4, C).astype(np.float32)
    out = cells.reshape(H, W, C).transpose(2, 0, 1)[None].astype(np.float32)
    return out
